# revision 6
# baseline (speedup 1.0000x reference)
"""LOCA kernel v3 for Trainium2, data-parallel over batch on 8 cores.

Per core (one batch element), softmax-weighted object reduction computed as
   g = r0 + (d1*u + d2*v) / (1 + u + v),   u = e^{d1}, v = e^{d2}
with d_k = r_k - r0 emitted directly by the conv (PE matmul over an
im2col'd fp8 feature window, DoubleRow mode), exp on ACT during PSUM
evacuation, the per-element d*u products on DVE (STT from PSUM), the
object sums via accumulating SWDGE DMAs, the division via ACT
Reciprocal, and the channel sum + r0 term (h-rows) + head weights in one
PE matmul.  Bilinear 8x upsample = two separable PE matmul passes.
"""

import sys

sys.path.insert(0, "/opt/trn_rl_repo")

import numpy as np
from contextlib import ExitStack

import concourse.bass as bass
import concourse.mybir as mybir
from concourse import bacc, tile
from concourse.bass_utils import run_bass_kernel_spmd

BS, C, H, W = 8, 256, 64, 64
STEPS, NO = 3, 3
RED = 8
HO, WO = H * RED, W * RED
HP, WP = H + 2, W + 4  # 66 x 68 padded window
FPLEN = HP * WP  # 4488
XWIN = (H - 1) * WP + W  # 4348: flat window length covering 64x64 out pixels
HW = H * W
NCORES = 8

F16 = mybir.dt.bfloat16
F32 = mybir.dt.float32
F8 = mybir.dt.float8e4
AF = mybir.ActivationFunctionType
ALU = mybir.AluOpType
PM = mybir.MatmulPerfMode

# pack geometry: 12 packs of 20 channels + 1 pack of 16
CPPS = [20] * 12 + [16]
NPACK = len(CPPS)
CH0 = [sum(CPPS[:i]) for i in range(NPACK)]
HSCALE = 64.0  # conv-lhs scaling of h rows (head divides back)
DSCALE = 4.0  # conv-lhs scaling of d rows (STT/exp divide back)
EPS_H = 2.0 ** -20

USE_DR = True  # fp8 DoubleRow conv
USE_SWDGE_SUMS = True  # den/num via accumulating SWDGE DMA; else PE selectors


def _pack_geom(p):
    cpp = CPPS[p]
    spp = cpp // 2
    nr = cpp * 6 + 3  # d rows + 3 h rows
    return cpp, spp, nr


def _bilinear_matrix(n_in, n_out):
    U = np.zeros((n_out, n_in), np.float64)
    s = n_in / n_out
    for i in range(n_out):
        c = (i + 0.5) * s - 0.5
        lo = int(np.floor(c))
        f = c - lo
        for idx, wt in ((lo, 1.0 - f), (lo + 1, f)):
            U[i, min(max(idx, 0), n_in - 1)] += wt
    return U


def _host_prep(f_e, all_prototypes, w_head, b_head):
    f_e = np.asarray(f_e, np.float32)
    ap = np.asarray(all_prototypes, np.float32)  # [S, 27, BS, C]
    w_head = np.asarray(w_head, np.float32)
    b_val = float(np.asarray(b_head).reshape(-1)[0])
    f8np = mybir.dt.np(F8)
    f16np = mybir.dt.np(F16)

    # padded feature map -> host-side 9x tap-replicated im2col source:
    # xrep[b, p, cl*9+t, sl, j] = fpad[b, ch0+cl+spp*sl, off(t)+j]
    fpad = np.zeros((BS, C, HP, WP), np.float32)
    fpad[:, :, 1 : 1 + H, 1 : 1 + W] = f_e
    fpad = fpad.reshape(BS, C, FPLEN)
    fpad2 = fpad.reshape(BS, C, HP, WP)
    xrep = np.zeros((BS, NPACK, 90, 2, HW), f16np)
    for p in range(NPACK):
        cpp, spp, nr = _pack_geom(p)
        for t in range(9):
            ki, kj = t // 3, t % 3
            win = fpad2[:, CH0[p] : CH0[p] + cpp, ki : ki + H, kj : kj + W]
            win = win.reshape(BS, 2, spp, HW)
            for cl in range(spp):
                xrep[:, p, cl * 9 + t, 0] = win[:, 0, cl]
                xrep[:, p, cl * 9 + t, 1] = win[:, 1, cl]

    # W[b, s, o, c, t]
    Wb = ap.reshape(STEPS, NO, 9, BS, C).transpose(3, 0, 1, 4, 2)

    # conv lhsT: [NPACK, 90, 2*128] fp8; row (cl_local, t) in slab sl maps to
    # channel ch0 + cl + spp*sl.  out col layout: cl*6 + s*2 + (o-1) for d
    # rows; cpp*6 + s for h rows.
    wconv = np.zeros((BS, NPACK, 90, 2, 128), np.float32)
    for p in range(NPACK):
        cpp, spp, nr = _pack_geom(p)
        for sl in range(2):
            for cl in range(spp):
                ch = CH0[p] + cl + spp * sl
                for t in range(9):
                    row = cl * 9 + t
                    for s in range(STEPS):
                        for o in (1, 2):
                            col = (o - 1) * cpp * 3 + (cl + spp * sl) * 3 + s
                            wconv[:, p, row, sl, col] = DSCALE * (
                                Wb[:, s, o, ch, t] - Wb[:, s, 0, ch, t]
                            )
                        colh = cpp * 6 + s
                        wconv[:, p, row, sl, colh] += (
                            HSCALE * w_head[ch] * Wb[:, s, 0, ch, t]
                        )
    wconv = wconv.reshape(BS, NPACK, 90, 256).astype(f16np)

    # exp scale vector per pack: [128,1] fp32: 1/DSCALE on d rows, tiny on h
    escale = np.zeros((NPACK, 128, 1), np.float32)
    for p in range(NPACK):
        cpp, spp, nr = _pack_geom(p)
        escale[p, : cpp * 6, 0] = 1.0 / DSCALE
        escale[p, cpp * 6 : nr, 0] = EPS_H
    # STT scale for B: d rows 1/DSCALE; h rows 1/DSCALE too (then head
    # divides by HSCALE/DSCALE... we keep h in B via same 1/DSCALE scalar);
    # python float scalar is uniform -> head lhs accounts for it.

    # head lhsT: per pair j (7 tiles incl last solo), rows = q rows of the
    # pair's packs stacked (60+60 or 48), cols = 3 steps; plus hacc lhs.
    # q row for (pack-local cl, s) at offset po*60 + cl*3 + s maps channel
    # CH0[pk]+cl, step s with weight w_head[ch].
    PAIRS = [(0, 1), (2, 3), (4, 5), (6, 7), (8, 9), (10, 11), (12,)]
    wq = np.zeros((len(PAIRS), 128, STEPS), np.float32)
    for j, pr in enumerate(PAIRS):
        off = 0
        for pk in pr:
            cpp = CPPS[pk]
            for cl in range(cpp):
                for s in range(STEPS):
                    wq[j, off + cl * 3 + s, s] = w_head[CH0[pk] + cl]
            off += cpp * 3
    # hacc rows: pack p's h rows at [p*3 + s] with B's extra 1/DSCALE applied:
    # B_h = (HSCALE*h*DSCALE... conv emits HSCALE*h*?? conv lhs h-weights are
    # HSCALE*w*W0; psum h-row = HSCALE*h; B = (psum * 1/DSCALE) * E_h where
    # E_h = exp(HSCALE*h*EPS_H) ~= 1.  So B_h ~= HSCALE*h/DSCALE.
    wh = np.zeros((NPACK * 3, STEPS), np.float32)
    for p in range(NPACK):
        for s in range(STEPS):
            wh[p * 3 + s, s] = DSCALE / HSCALE
    whq = np.zeros((128, STEPS), np.float32)
    whq[: NPACK * 3] = wh

    ut = _bilinear_matrix(H, HO).T.astype(f16np)  # [64, 512]
    eye = np.eye(128, dtype=f16np)

    in_maps = []
    for b in range(BS):
        in_maps.append(
            {
                "xrep": np.ascontiguousarray(xrep[b].reshape(NPACK, 90, 2 * HW)),
                "wconv": np.ascontiguousarray(wconv[b]),
                "escale": escale,
                "wq": wq.astype(f16np),
                "whq": whq.astype(f16np),
                "ut": ut,
                "eye": eye,
            }
        )
    return in_maps, b_val


def _build_nc(b_val: float) -> bass.Bass:
    nc = bacc.Bacc(None, target_bir_lowering=False)
    xrep_d = nc.declare_dram_parameter("xrep", [NPACK, 90, 2 * HW], F16, isOutput=False)
    wconv_d = nc.declare_dram_parameter("wconv", [NPACK, 90, 256], F16, isOutput=False)
    escale_d = nc.declare_dram_parameter("escale", [NPACK, 128, 1], F32, isOutput=False)
    wq_d = nc.declare_dram_parameter("wq", [7, 128, STEPS], F16, isOutput=False)
    whq_d = nc.declare_dram_parameter("whq", [128, STEPS], F16, isOutput=False)
    ut_d = nc.declare_dram_parameter("ut", [64, WO], F16, isOutput=False)
    eye_d = nc.declare_dram_parameter("eye", [128, 128], F16, isOutput=False)
    out_d = nc.declare_dram_parameter("out", [STEPS, HO, WO], F16, isOutput=True)

    with tile.TileContext(nc) as tc, ExitStack() as ctx:
        ctx.enter_context(
            nc.allow_low_precision(reason="bf16 softmax pipeline within tolerance")
        )
        const = ctx.enter_context(tc.tile_pool(name="const", bufs=1))
        xpool = ctx.enter_context(tc.tile_pool(name="xpool", bufs=2))
        ebpool = ctx.enter_context(tc.tile_pool(name="ebpool", bufs=2))
        denpool = ctx.enter_context(tc.tile_pool(name="denpool", bufs=3))
        vtpool = ctx.enter_context(tc.tile_pool(name="vtpool", bufs=2))
        numpool = ctx.enter_context(tc.tile_pool(name="numpool", bufs=7))
        hpool = ctx.enter_context(tc.tile_pool(name="hpool", bufs=1))
        opool = ctx.enter_context(tc.tile_pool(name="opool", bufs=2))

        # ---- constants (merged single DMAs) ----
        wc_all = const.tile([90, NPACK * 256], F16, tag="wc_all")
        nc.sync.dma_start(
            out=wc_all[:].rearrange("p (n f) -> p n f", n=NPACK),
            in_=wconv_d[:].transpose([1, 0, 2]),
        )
        es_all = const.tile([128, NPACK], F32, tag="es_all")
        nc.sync.dma_start(
            out=es_all[:].rearrange("p (n f) -> p n f", f=1),
            in_=escale_d[:].transpose([1, 0, 2]),
        )
        wqa = const.tile([128, 7 * STEPS], F16, tag="wqa")
        nc.sync.dma_start(
            out=wqa[:].rearrange("p (n f) -> p n f", n=7),
            in_=wq_d[:].transpose([1, 0, 2]),
        )
        wconv_sb = [wc_all[:].rearrange("p (n f) -> p n f", n=NPACK)[:, j] for j in range(NPACK)]
        escale_sb = [es_all[:, j : j + 1] for j in range(NPACK)]
        wq_sb = [wqa[:].rearrange("p (n f) -> p n f", n=7)[:, j] for j in range(7)]
        whq_sb = const.tile([128, STEPS], F16, tag="whq")
        nc.sync.dma_start(out=whq_sb[:], in_=whq_d[:])
        ut_sb = const.tile([64, WO], F16, tag="ut")
        nc.sync.dma_start(out=ut_sb[:], in_=ut_d[:])
        eye_sb = const.tile([128, 128], F16, tag="eye")
        nc.sync.dma_start(out=eye_sb[:], in_=eye_d[:])

        hacc = hpool.tile([NPACK * 3, HW], F16, tag="hacc")

        PAIRS = [(0, 1), (2, 3), (4, 5), (6, 7), (8, 9), (10, 11), (12,)]
        den_tiles, num_tiles = [], []

        with tc.tile_pool(name="ps_conv", bufs=2, space="PSUM") as ps_conv:
            for jp, pr in enumerate(PAIRS):
                den = denpool.tile([120, HW], F16, tag="den")
                num = numpool.tile([120, HW], F16, tag="num")
                vt = vtpool.tile([120, HW], F16, tag="vt")
                bt = vtpool.tile([120, HW], F16, tag="bt")
                den_tiles.append(den)
                num_tiles.append(num)
                roff = 0
                for pk in pr:
                    cpp, spp, nr = _pack_geom(pk)
                    nqr = cpp * 3
                    # ---- im2col: one DMA, [spp*9, 2, XWIN] fp8 ----
                    X = xpool.tile([90, 2 * HW], F16, tag="X")
                    half9 = spp * 9 // 2 * 2
                    nc.sync.dma_start(
                        out=X[0 : spp * 9, 0:HW],
                        in_=xrep_d[pk, 0 : spp * 9].rearrange(
                            "p (sl j) -> p sl j", sl=2
                        )[:, 0],
                    )
                    nc.scalar.dma_start(
                        out=X[0 : spp * 9, HW : 2 * HW],
                        in_=xrep_d[pk, 0 : spp * 9].rearrange(
                            "p (sl j) -> p sl j", sl=2
                        )[:, 1],
                    )
                    xw = X[:].rearrange("p (sl j) -> p sl j", sl=2)

                    wvs = wconv_sb[pk].rearrange("k (two m) -> k two m", two=2)
                    E = ebpool.tile([nr, HW], F16, tag="E")
                    B = ebpool.tile([nr, HW], F16, tag="B")
                    for half in range(2):
                        acc = ps_conv.tile([123, 2048], F32, tag="acc")
                        for q in range(4):
                            p0 = half * 2048 + q * 512
                            dst = acc[0:nr, q * 512 : (q + 1) * 512]
                            nc.tensor.matmul(
                                dst, wvs[0 : spp * 9, 0, 0:nr],
                                xw[0 : spp * 9, 0, p0 : p0 + 512],
                                start=True, stop=False,
                            )
                            nc.tensor.matmul(
                                dst, wvs[0 : spp * 9, 1, 0:nr],
                                xw[0 : spp * 9, 1, p0 : p0 + 512],
                                start=False, stop=True,
                            )
                        hs = slice(half * 2048, (half + 1) * 2048)
                        nc.scalar.activation(
                            E[:, hs], acc[0:nr], AF.Exp, scale=escale_sb[pk][0:nr, :]
                        )
                        nc.vector.scalar_tensor_tensor(
                            B[:, hs],
                            acc[0:nr],
                            1.0 / DSCALE,
                            E[:, hs],
                            op0=ALU.mult,
                            op1=ALU.mult,
                        )
                    # h rows -> hacc (plain copy DMA)
                    nc.sync.dma_start(
                        out=hacc[pk * 3 : pk * 3 + 3, :], in_=B[cpp * 6 : cpp * 6 + 3, :]
                    )
                    # object sums: align u/v blocks via plain DMA copies,
                    # then add on Pool (den) and DVE (num)
                    nc.sync.dma_start(
                        out=den[roff : roff + nqr, :], in_=E[0:nqr, :]
                    )
                    nc.sync.dma_start(
                        out=vt[roff : roff + nqr, :], in_=E[nqr : 2 * nqr, :]
                    )
                    nc.scalar.dma_start(
                        out=num[roff : roff + nqr, :], in_=B[0:nqr, :]
                    )
                    nc.scalar.dma_start(
                        out=bt[roff : roff + nqr, :], in_=B[nqr : 2 * nqr, :]
                    )
                    roff += nqr
                npair = roff
                nc.gpsimd.tensor_tensor(
                    den[0:npair, :], den[0:npair, :], vt[0:npair, :], op=ALU.add
                )
                nc.vector.tensor_tensor(
                    num[0:npair, :], num[0:npair, :], bt[0:npair, :], op=ALU.add
                )

        # ---- reciprocal phase (ACT, one table swap) + q (in place on num) ----
        q_tiles = []
        for jp, pr in enumerate(PAIRS):
            nrows = sum(CPPS[pk] * 3 for pk in pr)
            dt_ = den_tiles[jp]
            nc.vector.tensor_scalar(
                dt_[0:nrows], dt_[0:nrows], 1.0, 0.0, op0=ALU.add, op1=ALU.add
            )
            nc.vector.reciprocal(dt_[0:nrows], dt_[0:nrows])
            qt = num_tiles[jp]
            nc.vector.tensor_tensor(
                qt[0:nrows], qt[0:nrows], dt_[0:nrows], op=ALU.mult
            )
            q_tiles.append((qt, nrows))

        # ---- head + relu + upsample ----
        with tc.tile_pool(name="ps_tail", bufs=1, space="PSUM") as ps_tail:
            dm = hpool.tile([STEPS, HW], F16, tag="dm")
            for k in range(8):
                pd = ps_tail.tile([STEPS, 512], F32, tag="pd")
                for jp, (qt, nrows) in enumerate(q_tiles):
                    nc.tensor.matmul(
                        pd[:],
                        wq_sb[jp][0:nrows, :],
                        qt[0:nrows, k * 512 : (k + 1) * 512],
                        start=(jp == 0),
                        stop=False,
                    )
                nc.tensor.matmul(
                    pd[:],
                    whq_sb[0 : NPACK * 3],
                    hacc[:, k * 512 : (k + 1) * 512],
                    start=False,
                    stop=True,
                )
                # relu(x + b)
                nc.scalar.activation(
                    dm[:, k * 512 : (k + 1) * 512], pd[:], AF.Relu, bias=b_val
                )

            # bounce dm through the tail of out_d (rows 488:512 of step 2,
            # rewritten by the final upsample store afterwards)
            nc.sync.dma_start(
                out=out_d[2, 488:512, :].rearrange("(p a) x -> p (a x)", a=8),
                in_=dm[:],
            )
            for s in range(STEPS):
                dmY = opool.tile([64, 64], F16, tag="dmY")
                nc.sync.dma_start(
                    out=dmY[:],
                    in_=out_d[2, 488 + 8 * s : 496 + 8 * s, :].rearrange(
                        "(p a) x -> p (a x)", a=1
                    ).rearrange("p (y x) -> (p y) x", x=64),
                )
                psT0 = ps_tail.tile([64, 64], F16, tag="upsT")
                nc.tensor.transpose(psT0[:], dmY[:], eye_sb[0:64, 0:64])
                dmX = opool.tile([64, 64], F16, tag="dmX")
                nc.vector.tensor_copy(dmX[:], psT0[:])
                ps_h = ps_tail.tile([128, 512], F32, tag="ps_h")
                for xc in range(4):
                    nc.tensor.matmul(
                        ps_h[:, xc * 64 : (xc + 1) * 64],
                        ut_sb[:, xc * 128 : (xc + 1) * 128],
                        dmX[:],
                        start=True,
                        stop=True,
                    )
                h_sb = opool.tile([128, 256], F16, tag="h_sb")
                nc.scalar.activation(h_sb[:], ps_h[:, 0:256], AF.Copy)
                hyT = opool.tile([64, 512], F16, tag="hyT")
                for xc in range(4):
                    psTx = ps_tail.tile([64, 128], F16, tag="upsT2")
                    nc.tensor.transpose(
                        psTx[:], h_sb[:, xc * 64 : (xc + 1) * 64], eye_sb[:]
                    )
                    nc.vector.tensor_copy(hyT[:, xc * 128 : (xc + 1) * 128], psTx[:])
                for yc in range(4):
                    pv = ps_tail.tile([128, 512], F32, tag="pv")
                    nc.tensor.matmul(
                        pv[:],
                        ut_sb[:, yc * 128 : (yc + 1) * 128],
                        hyT[:],
                        start=True,
                        stop=True,
                    )
                    osb = opool.tile([128, 512], F16, tag="osb")
                    if yc % 2 == 0:
                        nc.scalar.activation(osb[:], pv[:], AF.Copy)
                    else:
                        nc.vector.tensor_copy(osb[:], pv[:])
                    nc.sync.dma_start(
                        out=out_d[s, yc * 128 : (yc + 1) * 128, :], in_=osb[:]
                    )

    nc.compile()
    return nc


_CACHE = {}


def _get_nc(b_val: float) -> bass.Bass:
    key = round(b_val, 12)
    if key not in _CACHE:
        _CACHE[key] = _build_nc(b_val)
    return _CACHE[key]


def kernel(f_e, all_prototypes, w_head, b_head):
    in_maps, b_val = _host_prep(f_e, all_prototypes, w_head, b_head)
    nc = _get_nc(b_val)
    res = run_bass_kernel_spmd(nc, in_maps, list(range(NCORES)), trace=False)
    outs = [
        np.asarray(res.results[b]["out"], np.float32).reshape(STEPS, 1, HO, WO)
        for b in range(BS)
    ]
    return np.stack(outs, axis=1)
